# revision 7
# baseline (speedup 1.0000x reference)
"""AgentAttention Trainium2 kernel (8 NeuronCores).

Sharding: core c -> (batch b = c//2, head-group g = c%2). Each core computes
its batch's QKV projections for its 8 heads (512 channels), both attention
stages, and a partial output projection over its 512 channels for all 4096
tokens. The host sums the two per-batch partials and adds the bias.

Layout strategy: activations flow channel-major ("transposed", [C, T]) so
every matmul contraction sits on the partition axis with no on-chip
transposes. Softmaxes skip max-subtraction (logits are O(6) for this
problem's distribution); row sums come from ones-columns/ones-matmuls and
the per-(head, token) stage-2 normalizer is broadcast across partitions
with a selection matmul. All matmul operands are float32r (~1e-4 relative
error, 4x the float32 PE rate at moving-dim >= 256).

Heads are processed in pairs: the two heads' [64, d] agent blocks sit in a
block-diagonal [128, 128] operand so every PE op runs with full 128-wide
contraction/output. Stage-1's attn @ v runs per pair against a [t, 258]
slab holding two pairs' v channels plus a ones column (the l1 row-sum
rides along as output column 256).

Emission is breadth-first and pass 2 is software-pipelined
(stage2(b) -> q-proj(b+1) -> y-proj(b)) so the in-order PE stream always
has dense matmul work while ACT/DVE chase the exp/copy chain.
"""

import os
import sys

import numpy as np
import ml_dtypes

for _p in ("/opt/trn_rl_repo", "/root/.axon_site/_ro/trn_rl_repo"):
    if os.path.isdir(_p) and _p not in sys.path:
        sys.path.append(_p)

import concourse.tile as tile
from concourse import bacc, mybir
from concourse.bass_utils import run_bass_kernel_spmd

f32 = mybir.dt.float32
f32r = mybir.dt.float32r
bf16 = mybir.dt.bfloat16
Exp = mybir.ActivationFunctionType.Exp

B, N, C = 4, 4096, 1024
H, D = 16, 64
G = 2                 # head-groups (2-way tensor parallel x 4-way batch parallel)
GC = C // G           # 512 channels per group
NPAIR = GC // 128     # 4 head-pairs per group
TB = 512              # token block
NB = N // TB
KO = C // 128         # k-tiles over the 1024 input channels
AVW = 258             # stage-1 A_v slab: 2 pairs x 128 channels + ones + pad
SCALE = D ** -0.5

_cache = {}


def _build():
    if "nc" in _cache:
        return _cache["nc"]
    nc = bacc.Bacc("TRN2", target_bir_lowering=False, debug=False)

    xT_d = nc.dram_tensor("xT", [C, N], f32, kind="ExternalInput")
    wq_d = nc.dram_tensor("wqT", [C, GC], f32, kind="ExternalInput")
    wk_d = nc.dram_tensor("wkT", [C, GC], f32, kind="ExternalInput")
    wv_d = nc.dram_tensor("wvT", [C, GC], f32, kind="ExternalInput")
    wp_d = nc.dram_tensor("wpT", [GC, C], f32, kind="ExternalInput")
    ab_d = nc.dram_tensor("ablk", [NPAIR, 128, 128], f32, kind="ExternalInput")
    ones2_d = nc.dram_tensor("ones2", [128, 2], f32, kind="ExternalInput")
    sel2_d = nc.dram_tensor("sel2", [2, 128], f32, kind="ExternalInput")
    zeros_d = nc.dram_tensor("zeros", [128, NPAIR * 128], f32, kind="ExternalInput")
    vones_d = nc.dram_tensor("vones", [128, 4, 2, 2], f32, kind="ExternalInput")
    y_d = nc.dram_tensor("y", [N, C], f32, kind="ExternalOutput")

    xT_r = xT_d.ap().bitcast(f32r).rearrange("(ko p) t -> p ko t", p=128)
    wq_r = wq_d.ap().bitcast(f32r).rearrange("(ko p) m -> p ko m", p=128)
    wk_r = wk_d.ap().bitcast(f32r).rearrange("(ko p) m -> p ko m", p=128)
    wv_r = wv_d.ap().bitcast(f32r).rearrange("(ko p) m -> p ko m", p=128)
    wp_r = wp_d.ap().bitcast(f32r).rearrange("(ko p) j -> p ko j", p=128)
    ab_r = ab_d.ap().bitcast(f32r).rearrange("h p a -> p h a")
    zeros_r = zeros_d.ap().bitcast(f32r).rearrange("p (h a) -> p h a", h=NPAIR)

    with tile.TileContext(nc) as tc:
        with (
            tc.tile_pool(name="wpool", bufs=1) as wpool,
            tc.tile_pool(name="cpool", bufs=1) as cpool,
            tc.tile_pool(name="accp", bufs=1) as accp,
        ):
            # weights + constants ride the ACT HWDGE ring so the SP ring can
            # start streaming x immediately
            wk_t = wpool.tile([128, KO, GC], f32r)
            wv_t = wpool.tile([128, KO, GC], f32r)
            wq_t = wpool.tile([128, KO, GC], f32r)
            wp_t = wpool.tile([128, NPAIR, C], f32r)
            for ko in range(KO):
                nc.scalar.dma_start(wk_t[:, ko, :], wk_r[:, ko, :])
            nc.scalar.dma_start(wv_t[:], wv_r)
            ab_t = cpool.tile([128, NPAIR, 128], f32r)
            nc.scalar.dma_start(ab_t[:], ab_r)
            ones2_t = cpool.tile([128, 2], f32r)
            nc.scalar.dma_start(ones2_t[:], ones2_d.ap().bitcast(f32r))
            sel2_t = cpool.tile([2, 128], f32r)
            nc.scalar.dma_start(sel2_t[:], sel2_d.ap().bitcast(f32r))
            avacc = accp.tile([128, NPAIR, AVW], f32)
            avm = accp.tile([128, NPAIR, 128], f32r)
            nc.scalar.dma_start(avm[:], zeros_r)

            # ---------------- pass 1: k/v projections + stage-1 attention ----
            with (
                tc.tile_pool(name="xp1", bufs=2) as xp1,
                tc.tile_pool(name="kvp", bufs=2) as kvp,
                tc.tile_pool(name="pp1", bufs=8) as pp1,
                tc.tile_pool(name="psA", bufs=2, space="PSUM") as psA,
                tc.tile_pool(name="psB", bufs=4, space="PSUM") as psB,
                tc.tile_pool(name="psC", bufs=2, space="PSUM") as psC,
            ):
                for tb in range(NB):
                    xblk = xp1.tile([128, KO, TB], f32r, tag="xblk")
                    xsrc = xT_r[:, :, tb * TB:(tb + 1) * TB]
                    if tb == 0:
                        for ko in range(KO):
                            nc.sync.dma_start(xblk[:, ko, :], xsrc[:, ko, :])
                    else:
                        nc.sync.dma_start(xblk[:], xsrc)

                    # kT: [pair-channel, pair, tokens]
                    kT = kvp.tile([128, NPAIR, TB], f32r, tag="kT")
                    for m in range(NPAIR):
                        psk = psA.tile([128, TB], f32, tag="proj")
                        for ko in range(KO):
                            nc.tensor.matmul(
                                psk[:],
                                lhsT=wk_t[:, ko, m * 128:(m + 1) * 128],
                                rhs=xblk[:, ko, :],
                                start=(ko == 0), stop=(ko == KO - 1),
                            )
                        nc.scalar.copy(kT[:, m, :], psk[:])

                    # v slabs: [token-in-chunk, chunk, pair-group, 256 ch|1|0]
                    vt = kvp.tile([128, 4, 2, AVW], f32r, tag="vt")
                    nc.sync.dma_start(vt[:, :, :, 256:258], vones_d.ap().bitcast(f32r))
                    for mt in range(4):
                        psv = psA.tile([128, GC], f32, tag="proj")
                        for ko in range(KO):
                            nc.tensor.matmul(
                                psv[:],
                                lhsT=xblk[:, ko, mt * 128:(mt + 1) * 128],
                                rhs=wv_t[:, ko, :],
                                start=(ko == 0), stop=(ko == KO - 1),
                            )
                        nc.vector.tensor_copy(vt[:, mt, 0, 0:256], psv[:, 0:256])
                        nc.vector.tensor_copy(vt[:, mt, 1, 0:256], psv[:, 256:512])

                    # stage 1: agents attend over this block's keys.
                    # breadth-first: the 4 s1 matmuls run dense while ACT exps
                    # drain them, then the 4 av matmuls accumulate.
                    for hp in range(NPAIR):
                        pts = []
                        for ch in range(4):
                            pss = psB.tile([128, 128], f32, tag="s1")
                            nc.tensor.matmul(
                                pss[:],
                                lhsT=kT[:, hp, ch * 128:(ch + 1) * 128],
                                rhs=ab_t[:, hp, :],
                                start=True, stop=True,
                            )
                            pt = pp1.tile([128, 128], f32r, tag="p1")
                            nc.scalar.activation(pt[:], pss[:], Exp, scale=SCALE)
                            pts.append(pt)
                        psav = psC.tile([128, AVW], f32, tag="av")
                        for ch in range(4):
                            nc.tensor.matmul(
                                psav[:], lhsT=pts[ch][:],
                                rhs=vt[:, ch, hp // 2, :],
                                start=(ch == 0), stop=(ch == 3),
                            )
                        if tb == 0:
                            nc.vector.tensor_copy(avacc[:, hp, :], psav[:])
                        else:
                            nc.vector.tensor_add(avacc[:, hp, :], avacc[:, hp, :], psav[:])

                    if tb == 2:
                        # q/p weights arrive mid-pass-1: HBM is quiet here and
                        # they are not needed until pass 2
                        nc.scalar.dma_start(wq_t[:], wq_r)
                        nc.scalar.dma_start(wp_t[:], wp_r)

            # ---------------- pass 2: q projection + stage-2 + output proj ---
            with (
                tc.tile_pool(name="xp2", bufs=2) as xp2,
                tc.tile_pool(name="qp", bufs=2) as qp,
                tc.tile_pool(name="pp2", bufs=4) as pp2,
                tc.tile_pool(name="op", bufs=2) as op,
                tc.tile_pool(name="yp", bufs=4) as yp,
                tc.tile_pool(name="psD", bufs=2, space="PSUM") as psD,
                tc.tile_pool(name="psY", bufs=2, space="PSUM") as psY,
                tc.tile_pool(name="psSBO", bufs=3, space="PSUM") as psSBO,
                tc.tile_pool(name="psF", bufs=1, space="PSUM") as psF,
            ):
                def emit_qproj(tb):
                    xblk = xp2.tile([128, KO, TB], f32r, tag="xblk2")
                    nc.sync.dma_start(xblk[:], xT_r[:, :, tb * TB:(tb + 1) * TB])
                    qT = qp.tile([128, NPAIR, TB], f32r, tag="qT")
                    for m in range(NPAIR):
                        psq = psD.tile([128, TB], f32, tag="proj2")
                        for ko in range(KO):
                            nc.tensor.matmul(
                                psq[:],
                                lhsT=wq_t[:, ko, m * 128:(m + 1) * 128],
                                rhs=xblk[:, ko, :],
                                start=(ko == 0), stop=(ko == KO - 1),
                            )
                        nc.vector.tensor_copy(qT[:, m, :], psq[:])
                    return qT

                def emit_stage2(qT):
                    outT = op.tile([128, NPAIR, TB], f32r, tag="outT")
                    p2s, r2rs = [], []
                    for hp in range(NPAIR):
                        ps2 = psSBO.tile([128, TB], f32, tag="sbo")
                        nc.tensor.matmul(ps2[:], lhsT=ab_t[:, hp, :],
                                         rhs=qT[:, hp, :], start=True, stop=True)
                        p2 = pp2.tile([128, TB], f32r, tag="p2")
                        nc.scalar.activation(p2[:], ps2[:], Exp, scale=SCALE)
                        p2s.append(p2)
                    for hp in range(NPAIR):
                        psl2 = psF.tile([2, TB], f32, tag="l2")
                        nc.tensor.matmul(psl2[:], lhsT=ones2_t[:], rhs=p2s[hp][:],
                                         start=True, stop=True)
                        r2 = pp2.tile([2, TB], f32, tag="r2")
                        r2w = pp2.tile([2, TB], f32, tag="r2w")
                        nc.vector.reciprocal_approx_accurate(r2[:], psl2[:], r2w[:])
                        r2r = pp2.tile([2, TB], f32r, tag="r2r")
                        nc.vector.tensor_copy(r2r[:], r2[:])
                        r2rs.append(r2r)
                    for hp in range(NPAIR):
                        pso = psSBO.tile([128, TB], f32, tag="sbo")
                        nc.tensor.matmul(pso[:], lhsT=avm[:, hp, :], rhs=p2s[hp][:],
                                         start=True, stop=True)
                        psbc = psSBO.tile([128, TB], f32, tag="sbo")
                        nc.tensor.matmul(psbc[:], lhsT=sel2_t[:], rhs=r2rs[hp][:],
                                         start=True, stop=True)
                        bcs = pp2.tile([128, TB], f32, tag="bcs")
                        nc.scalar.copy(bcs[:], psbc[:])
                        with nc.allow_low_precision(reason="f32r rounding"):
                            nc.vector.tensor_mul(outT[:, hp, :], pso[:], bcs[:])
                    return outT

                def emit_yproj(tb, outT):
                    for tt in range(4):
                        for jn in range(2):
                            psy = psY.tile([128, 512], f32, tag="yproj")
                            for hp in range(NPAIR):
                                nc.tensor.matmul(
                                    psy[:],
                                    lhsT=outT[:, hp, tt * 128:(tt + 1) * 128],
                                    rhs=wp_t[:, hp, jn * 512:(jn + 1) * 512],
                                    start=(hp == 0), stop=(hp == NPAIR - 1),
                                )
                            ys = yp.tile([128, 512], f32, tag="ys")
                            if jn == 0:
                                nc.scalar.copy(ys[:], psy[:])
                            else:
                                nc.vector.tensor_copy(ys[:], psy[:])
                            row = tb * TB + tt * 128
                            nc.sync.dma_start(
                                y_d.ap()[row:row + 128, jn * 512:(jn + 1) * 512],
                                ys[:])

                qT = emit_qproj(0)

                # normalize agent_v, mask into block-diagonal f32r operand
                # (overlaps the q projection of block 0 on DVE)
                for hp in range(NPAIR):
                    hi = hp % 2
                    r1 = pp2.tile([128, 1], f32, tag="r1")
                    r1s = pp2.tile([128, 1], f32, tag="r1s")
                    nc.vector.reciprocal_approx_accurate(
                        r1[:], avacc[:, hp, 256:257], r1s[:])
                    with nc.allow_low_precision(reason="f32r rounding"):
                        nc.vector.tensor_scalar_mul(
                            avm[0:64, hp, 0:64],
                            avacc[0:64, hp, hi * 128:hi * 128 + 64],
                            r1[0:64, :])
                        nc.vector.tensor_scalar_mul(
                            avm[64:128, hp, 64:128],
                            avacc[64:128, hp, hi * 128 + 64:hi * 128 + 128],
                            r1[64:128, :])

                for tb in range(NB):
                    outT = emit_stage2(qT)
                    if tb + 1 < NB:
                        qT = emit_qproj(tb + 1)
                    emit_yproj(tb, outT)

    nc.compile()
    _cache["nc"] = nc
    return nc


def _consts():
    ones2 = np.zeros((128, 2), np.float32)
    ones2[0:64, 0] = 1.0
    ones2[64:128, 1] = 1.0
    sel2 = np.zeros((2, 128), np.float32)
    sel2[0, 0:64] = 1.0
    sel2[1, 64:128] = 1.0
    zeros = np.zeros((128, NPAIR * 128), np.float32)
    vones = np.zeros((128, 4, 2, 2), np.float32)
    vones[:, :, :, 0] = 1.0
    return ones2, sel2, zeros, vones


def _in_maps(x, agent_tokens, Wq, Wk, Wv, Wp):
    ones2, sel2, zeros, vones = _consts()
    xT = [np.ascontiguousarray(x[b].T) for b in range(B)]
    wqT = [np.ascontiguousarray(Wq[g * GC:(g + 1) * GC, :].T) for g in range(G)]
    wkT = [np.ascontiguousarray(Wk[g * GC:(g + 1) * GC, :].T) for g in range(G)]
    wvT = [np.ascontiguousarray(Wv[g * GC:(g + 1) * GC, :].T) for g in range(G)]
    wpT = [np.ascontiguousarray(Wp[:, g * GC:(g + 1) * GC].T) for g in range(G)]
    in_maps = []
    for core in range(8):
        b, g = core // 2, core % 2
        aT = agent_tokens[b][:, g * GC:(g + 1) * GC].T   # [512, 64]
        ablk = np.zeros((NPAIR, 128, 128), np.float32)
        for hp in range(NPAIR):
            ablk[hp, 0:64, 0:64] = aT[hp * 128:hp * 128 + 64, :]
            ablk[hp, 64:128, 64:128] = aT[hp * 128 + 64:hp * 128 + 128, :]
        in_maps.append({
            "xT": xT[b], "wqT": wqT[g], "wkT": wkT[g], "wvT": wvT[g],
            "wpT": wpT[g], "ablk": ablk, "ones2": ones2, "sel2": sel2,
            "zeros": zeros, "vones": vones,
        })
    return in_maps


def kernel(x, agent_tokens, Wq, Wk, Wv, Wp, bp):
    x = np.asarray(x, dtype=np.float32)
    agent_tokens = np.asarray(agent_tokens, dtype=np.float32)
    Wq = np.asarray(Wq, dtype=np.float32)
    Wk = np.asarray(Wk, dtype=np.float32)
    Wv = np.asarray(Wv, dtype=np.float32)
    Wp = np.asarray(Wp, dtype=np.float32)
    bp = np.asarray(bp, dtype=np.float32)

    nc = _build()
    in_maps = _in_maps(x, agent_tokens, Wq, Wk, Wv, Wp)
    res = run_bass_kernel_spmd(nc, in_maps, list(range(8)))

    y = np.empty((B, N, C), np.float32)
    for b in range(B):
        y[b] = res.results[2 * b]["y"] + res.results[2 * b + 1]["y"] + bp[None, :]
    return y


if __name__ == "__main__":
    rng = np.random.default_rng(0)
    ins = {
        "x": rng.standard_normal((B, N, C)).astype(np.float32),
        "agent_tokens": rng.standard_normal((B, 64, C)).astype(np.float32),
        "Wq": (rng.standard_normal((C, C)) * C ** -0.5).astype(np.float32),
        "Wk": (rng.standard_normal((C, C)) * C ** -0.5).astype(np.float32),
        "Wv": (rng.standard_normal((C, C)) * C ** -0.5).astype(np.float32),
        "Wp": (rng.standard_normal((C, C)) * C ** -0.5).astype(np.float32),
        "bp": (rng.standard_normal((C,)) * 0.01).astype(np.float32),
    }
    out = kernel(**ins)
    print("out", out.shape, out.dtype)


# revision 11
# speedup vs baseline: 1.0017x; 1.0017x over previous
"""AgentAttention Trainium2 kernel (8 NeuronCores).

Sharding: core c -> (batch b = c//2, head-group g = c%2). Each core computes
its batch's QKV projections for its 8 heads (512 channels), both attention
stages, and a partial output projection over its 512 channels for all 4096
tokens. The host sums the two per-batch partials and adds the bias.

Layout strategy: activations flow channel-major ("transposed", [C, T]) so
every matmul contraction sits on the partition axis with no on-chip
transposes. Softmaxes skip max-subtraction (logits are O(6) for this
problem's distribution); row sums come from ones-columns/ones-matmuls and
the per-(head, token) stage-2 normalizer is broadcast across partitions
with a selection matmul. All matmul operands are float32r (~1e-4 relative
error, 4x the float32 PE rate at moving-dim >= 256).

Heads are processed in pairs: the two heads' [64, d] agent blocks sit in a
block-diagonal [128, 128] operand so every PE op runs with full 128-wide
contraction/output. Stage-1's attn @ v runs per pair against a [t, 258]
slab holding two pairs' v channels plus a ones column (the l1 row-sum
rides along as output column 256).

Emission is breadth-first and pass 2 is software-pipelined
(stage2(b) -> q-proj(b+1) -> y-proj(b)) so the in-order PE stream always
has dense matmul work while ACT/DVE chase the exp/copy chain.
"""

import os
import sys

import numpy as np
import ml_dtypes

for _p in ("/opt/trn_rl_repo", "/root/.axon_site/_ro/trn_rl_repo"):
    if os.path.isdir(_p) and _p not in sys.path:
        sys.path.append(_p)

import concourse.tile as tile
from concourse import bacc, mybir
from concourse.bass_utils import run_bass_kernel_spmd

f32 = mybir.dt.float32
f32r = mybir.dt.float32r
bf16 = mybir.dt.bfloat16
Exp = mybir.ActivationFunctionType.Exp

B, N, C = 4, 4096, 1024
H, D = 16, 64
G = 2                 # head-groups (2-way tensor parallel x 4-way batch parallel)
GC = C // G           # 512 channels per group
NPAIR = GC // 128     # 4 head-pairs per group
TB = 512              # token block
NB = N // TB
KO = C // 128         # k-tiles over the 1024 input channels
AVW = 258             # stage-1 A_v slab: 2 pairs x 128 channels + ones + pad
SCALE = D ** -0.5

_cache = {}


def _build():
    if "nc" in _cache:
        return _cache["nc"]
    nc = bacc.Bacc("TRN2", target_bir_lowering=False, debug=False)

    xT_d = nc.dram_tensor("xT", [C, N], f32, kind="ExternalInput")
    wq_d = nc.dram_tensor("wqT", [C, GC], f32, kind="ExternalInput")
    wk_d = nc.dram_tensor("wkT", [C, GC], f32, kind="ExternalInput")
    wv_d = nc.dram_tensor("wvT", [C, GC], f32, kind="ExternalInput")
    wp_d = nc.dram_tensor("wpT", [GC, C], f32, kind="ExternalInput")
    ab_d = nc.dram_tensor("ablk", [NPAIR, 128, 128], f32, kind="ExternalInput")
    ones2_d = nc.dram_tensor("ones2", [128, 2], f32, kind="ExternalInput")
    sel2_d = nc.dram_tensor("sel2", [2, 128], f32, kind="ExternalInput")
    zeros_d = nc.dram_tensor("zeros", [128, NPAIR * 128], f32, kind="ExternalInput")
    vones_d = nc.dram_tensor("vones", [128, 4, 2, 2], f32, kind="ExternalInput")
    ident_d = nc.dram_tensor("ident", [128, 128], f32, kind="ExternalInput")
    y_d = nc.dram_tensor("y", [N, C], f32, kind="ExternalOutput")

    xT_r = xT_d.ap().bitcast(f32r).rearrange("(ko p) t -> p ko t", p=128)
    wq_r = wq_d.ap().bitcast(f32r).rearrange("(ko p) m -> p ko m", p=128)
    wk_r = wk_d.ap().bitcast(f32r).rearrange("(ko p) m -> p ko m", p=128)
    wv_r = wv_d.ap().bitcast(f32r).rearrange("(ko p) m -> p ko m", p=128)
    wp_r = wp_d.ap().bitcast(f32r).rearrange("(ko p) j -> p ko j", p=128)
    ab_r = ab_d.ap().bitcast(f32r).rearrange("h p a -> p h a")
    zeros_r = zeros_d.ap().bitcast(f32r).rearrange("p (h a) -> p h a", h=NPAIR)

    with tile.TileContext(nc) as tc:
        with (
            tc.tile_pool(name="wpool", bufs=1) as wpool,
            tc.tile_pool(name="cpool", bufs=1) as cpool,
            tc.tile_pool(name="accp", bufs=1) as accp,
        ):
            # weights + constants ride the ACT HWDGE ring so the SP ring can
            # start streaming x immediately
            wk_t = wpool.tile([128, KO, GC], f32r)
            wv_t = wpool.tile([128, KO, GC], f32r)
            wq_t = wpool.tile([128, KO, GC], f32r)
            wp_t = wpool.tile([128, NPAIR, C], f32r)
            for ko in range(KO):
                nc.scalar.dma_start(wk_t[:, ko, :], wk_r[:, ko, :])
            nc.scalar.dma_start(wv_t[:], wv_r)
            ab_t = cpool.tile([128, NPAIR, 128], f32r)
            nc.scalar.dma_start(ab_t[:], ab_r)
            ones2_t = cpool.tile([128, 2], f32r)
            nc.scalar.dma_start(ones2_t[:], ones2_d.ap().bitcast(f32r))
            sel2_t = cpool.tile([2, 128], f32r)
            nc.scalar.dma_start(sel2_t[:], sel2_d.ap().bitcast(f32r))
            ident_t = cpool.tile([128, 128], f32r)
            nc.scalar.dma_start(ident_t[:], ident_d.ap().bitcast(f32r))
            avacc = accp.tile([128, NPAIR, AVW], f32)
            avm = accp.tile([128, NPAIR, 128], f32r)
            nc.scalar.dma_start(avm[:], zeros_r)

            # ---------------- pass 1: k/v projections + stage-1 attention ----
            with (
                tc.tile_pool(name="xp1", bufs=2) as xp1,
                tc.tile_pool(name="kvp", bufs=2) as kvp,
                tc.tile_pool(name="pp1", bufs=8) as pp1,
                tc.tile_pool(name="psA", bufs=2, space="PSUM") as psA,
                tc.tile_pool(name="psB", bufs=2, space="PSUM") as psB,
                tc.tile_pool(name="psC", bufs=2, space="PSUM") as psC,
            ):
                for tb in range(NB):
                    xblk = xp1.tile([128, KO, TB], f32r, tag="xblk")
                    xsrc = xT_r[:, :, tb * TB:(tb + 1) * TB]
                    if tb == 0:
                        for ko in range(KO):
                            nc.sync.dma_start(xblk[:, ko, :], xsrc[:, ko, :])
                    else:
                        nc.sync.dma_start(xblk[:], xsrc)

                    # kT: [pair-channel, pair, tokens]
                    kT = kvp.tile([128, NPAIR, TB], f32r, tag="kT")
                    for m in range(NPAIR):
                        psk = psA.tile([128, TB], f32, tag="proj")
                        for ko in range(KO):
                            nc.tensor.matmul(
                                psk[:],
                                lhsT=wk_t[:, ko, m * 128:(m + 1) * 128],
                                rhs=xblk[:, ko, :],
                                start=(ko == 0), stop=(ko == KO - 1),
                            )
                        nc.scalar.copy(kT[:, m, :], psk[:])

                    # v slabs: [token-in-chunk, chunk, pair-group, 256 ch|1|0]
                    vt = kvp.tile([128, 4, 2, AVW], f32r, tag="vt")
                    nc.sync.dma_start(vt[:, :, :, 256:258], vones_d.ap().bitcast(f32r))
                    for mt in range(4):
                        psv = psA.tile([128, GC], f32, tag="proj")
                        for ko in range(KO):
                            nc.tensor.matmul(
                                psv[:],
                                lhsT=xblk[:, ko, mt * 128:(mt + 1) * 128],
                                rhs=wv_t[:, ko, :],
                                start=(ko == 0), stop=(ko == KO - 1),
                            )
                        nc.vector.tensor_copy(vt[:, mt, 0, 0:256], psv[:, 0:256])
                        nc.vector.tensor_copy(vt[:, mt, 1, 0:256], psv[:, 256:512])

                    # stage 1: agents attend over this block's keys.
                    # one wide [a, t] logits matmul, exp on ACT, then PE
                    # transposes to [t, a] chunks feeding the A_v accumulation
                    for hp in range(NPAIR):
                        psT = psB.tile([128, TB], f32, tag="s1T")
                        nc.tensor.matmul(psT[:], lhsT=ab_t[:, hp, :],
                                         rhs=kT[:, hp, :], start=True, stop=True)
                        p1T = pp1.tile([128, TB], f32r, tag="p1T")
                        nc.scalar.activation(p1T[:], psT[:], Exp, scale=SCALE)
                        pts = []
                        for ch in range(4):
                            psX = psB.tile([128, 128], f32r, tag="ptr")
                            nc.tensor.transpose(
                                psX[:], p1T[:, ch * 128:(ch + 1) * 128], ident_t[:])
                            pt = pp1.tile([128, 128], f32r, tag="p1")
                            nc.vector.tensor_copy(pt[:], psX[:])
                            pts.append(pt)
                        psav = psC.tile([128, AVW], f32, tag="av")
                        for ch in range(4):
                            nc.tensor.matmul(
                                psav[:], lhsT=pts[ch][:],
                                rhs=vt[:, ch, hp // 2, :],
                                start=(ch == 0), stop=(ch == 3),
                            )
                        if tb == 0:
                            nc.vector.tensor_copy(avacc[:, hp, :], psav[:])
                        else:
                            nc.vector.tensor_add(avacc[:, hp, :], avacc[:, hp, :], psav[:])

                    if tb == 2:
                        # q/p weights arrive mid-pass-1: HBM is quiet here and
                        # they are not needed until pass 2
                        nc.scalar.dma_start(wq_t[:], wq_r)
                        nc.scalar.dma_start(wp_t[:], wp_r)

            # ---------------- pass 2: q projection + stage-2 + output proj ---
            with (
                tc.tile_pool(name="xp2", bufs=2) as xp2,
                tc.tile_pool(name="qp", bufs=2) as qp,
                tc.tile_pool(name="pp2", bufs=4) as pp2,
                tc.tile_pool(name="op", bufs=2) as op,
                tc.tile_pool(name="yp", bufs=4) as yp,
                tc.tile_pool(name="psD", bufs=2, space="PSUM") as psD,
                tc.tile_pool(name="psY", bufs=2, space="PSUM") as psY,
                tc.tile_pool(name="psSBO", bufs=3, space="PSUM") as psSBO,
                tc.tile_pool(name="psF", bufs=1, space="PSUM") as psF,
            ):
                def emit_qproj(tb):
                    xblk = xp2.tile([128, KO, TB], f32r, tag="xblk2")
                    nc.sync.dma_start(xblk[:], xT_r[:, :, tb * TB:(tb + 1) * TB])
                    qT = qp.tile([128, NPAIR, TB], f32r, tag="qT")
                    for m in range(NPAIR):
                        psq = psD.tile([128, TB], f32, tag="proj2")
                        for ko in range(KO):
                            nc.tensor.matmul(
                                psq[:],
                                lhsT=wq_t[:, ko, m * 128:(m + 1) * 128],
                                rhs=xblk[:, ko, :],
                                start=(ko == 0), stop=(ko == KO - 1),
                            )
                        nc.vector.tensor_copy(qT[:, m, :], psq[:])
                    return qT

                def emit_stage2(qT):
                    outT = op.tile([128, NPAIR, TB], f32r, tag="outT")
                    p2s, r2rs = [], []
                    for hp in range(NPAIR):
                        ps2 = psSBO.tile([128, TB], f32, tag="sbo")
                        nc.tensor.matmul(ps2[:], lhsT=ab_t[:, hp, :],
                                         rhs=qT[:, hp, :], start=True, stop=True)
                        p2 = pp2.tile([128, TB], f32r, tag="p2")
                        nc.scalar.activation(p2[:], ps2[:], Exp, scale=SCALE)
                        p2s.append(p2)
                    for hp in range(NPAIR):
                        psl2 = psF.tile([2, TB], f32, tag="l2")
                        nc.tensor.matmul(psl2[:], lhsT=ones2_t[:], rhs=p2s[hp][:],
                                         start=True, stop=True)
                        r2 = pp2.tile([2, TB], f32, tag="r2")
                        r2w = pp2.tile([2, TB], f32, tag="r2w")
                        nc.vector.reciprocal_approx_accurate(r2[:], psl2[:], r2w[:])
                        r2r = pp2.tile([2, TB], f32r, tag="r2r")
                        nc.vector.tensor_copy(r2r[:], r2[:])
                        r2rs.append(r2r)
                    for hp in range(NPAIR):
                        pso = psSBO.tile([128, TB], f32, tag="sbo")
                        nc.tensor.matmul(pso[:], lhsT=avm[:, hp, :], rhs=p2s[hp][:],
                                         start=True, stop=True)
                        psbc = psSBO.tile([128, TB], f32, tag="sbo")
                        nc.tensor.matmul(psbc[:], lhsT=sel2_t[:], rhs=r2rs[hp][:],
                                         start=True, stop=True)
                        bcs = pp2.tile([128, TB], f32, tag="bcs")
                        nc.scalar.copy(bcs[:], psbc[:])
                        with nc.allow_low_precision(reason="f32r rounding"):
                            nc.vector.tensor_mul(outT[:, hp, :], pso[:], bcs[:])
                    return outT

                def emit_yproj(tb, outT):
                    for tt in range(4):
                        for jn in range(2):
                            psy = psY.tile([128, 512], f32, tag="yproj")
                            for hp in range(NPAIR):
                                nc.tensor.matmul(
                                    psy[:],
                                    lhsT=outT[:, hp, tt * 128:(tt + 1) * 128],
                                    rhs=wp_t[:, hp, jn * 512:(jn + 1) * 512],
                                    start=(hp == 0), stop=(hp == NPAIR - 1),
                                )
                            ys = yp.tile([128, 512], f32, tag="ys")
                            if jn == 0:
                                nc.scalar.copy(ys[:], psy[:])
                            else:
                                nc.vector.tensor_copy(ys[:], psy[:])
                            row = tb * TB + tt * 128
                            nc.sync.dma_start(
                                y_d.ap()[row:row + 128, jn * 512:(jn + 1) * 512],
                                ys[:])

                qT = emit_qproj(0)

                # normalize agent_v, mask into block-diagonal f32r operand
                # (overlaps the q projection of block 0 on DVE)
                for hp in range(NPAIR):
                    hi = hp % 2
                    r1 = pp2.tile([128, 1], f32, tag="r1")
                    r1s = pp2.tile([128, 1], f32, tag="r1s")
                    nc.vector.reciprocal_approx_accurate(
                        r1[:], avacc[:, hp, 256:257], r1s[:])
                    with nc.allow_low_precision(reason="f32r rounding"):
                        nc.vector.tensor_scalar_mul(
                            avm[0:64, hp, 0:64],
                            avacc[0:64, hp, hi * 128:hi * 128 + 64],
                            r1[0:64, :])
                        nc.vector.tensor_scalar_mul(
                            avm[64:128, hp, 64:128],
                            avacc[64:128, hp, hi * 128 + 64:hi * 128 + 128],
                            r1[64:128, :])

                for tb in range(NB):
                    outT = emit_stage2(qT)
                    if tb + 1 < NB:
                        qT = emit_qproj(tb + 1)
                    emit_yproj(tb, outT)

    nc.compile()
    _cache["nc"] = nc
    return nc


def _consts():
    ones2 = np.zeros((128, 2), np.float32)
    ones2[0:64, 0] = 1.0
    ones2[64:128, 1] = 1.0
    sel2 = np.zeros((2, 128), np.float32)
    sel2[0, 0:64] = 1.0
    sel2[1, 64:128] = 1.0
    zeros = np.zeros((128, NPAIR * 128), np.float32)
    vones = np.zeros((128, 4, 2, 2), np.float32)
    vones[:, :, :, 0] = 1.0
    ident = np.eye(128, dtype=np.float32)
    return ones2, sel2, zeros, vones, ident


def _in_maps(x, agent_tokens, Wq, Wk, Wv, Wp):
    ones2, sel2, zeros, vones, ident = _consts()
    xT = [np.ascontiguousarray(x[b].T) for b in range(B)]
    wqT = [np.ascontiguousarray(Wq[g * GC:(g + 1) * GC, :].T) for g in range(G)]
    wkT = [np.ascontiguousarray(Wk[g * GC:(g + 1) * GC, :].T) for g in range(G)]
    wvT = [np.ascontiguousarray(Wv[g * GC:(g + 1) * GC, :].T) for g in range(G)]
    wpT = [np.ascontiguousarray(Wp[:, g * GC:(g + 1) * GC].T) for g in range(G)]
    in_maps = []
    for core in range(8):
        b, g = core // 2, core % 2
        aT = agent_tokens[b][:, g * GC:(g + 1) * GC].T   # [512, 64]
        ablk = np.zeros((NPAIR, 128, 128), np.float32)
        for hp in range(NPAIR):
            ablk[hp, 0:64, 0:64] = aT[hp * 128:hp * 128 + 64, :]
            ablk[hp, 64:128, 64:128] = aT[hp * 128 + 64:hp * 128 + 128, :]
        in_maps.append({
            "xT": xT[b], "wqT": wqT[g], "wkT": wkT[g], "wvT": wvT[g],
            "wpT": wpT[g], "ablk": ablk, "ones2": ones2, "sel2": sel2,
            "zeros": zeros, "vones": vones, "ident": ident,
        })
    return in_maps


def kernel(x, agent_tokens, Wq, Wk, Wv, Wp, bp):
    x = np.asarray(x, dtype=np.float32)
    agent_tokens = np.asarray(agent_tokens, dtype=np.float32)
    Wq = np.asarray(Wq, dtype=np.float32)
    Wk = np.asarray(Wk, dtype=np.float32)
    Wv = np.asarray(Wv, dtype=np.float32)
    Wp = np.asarray(Wp, dtype=np.float32)
    bp = np.asarray(bp, dtype=np.float32)

    nc = _build()
    in_maps = _in_maps(x, agent_tokens, Wq, Wk, Wv, Wp)
    res = run_bass_kernel_spmd(nc, in_maps, list(range(8)))

    y = np.empty((B, N, C), np.float32)
    for b in range(B):
        y[b] = res.results[2 * b]["y"] + res.results[2 * b + 1]["y"] + bp[None, :]
    return y


if __name__ == "__main__":
    rng = np.random.default_rng(0)
    ins = {
        "x": rng.standard_normal((B, N, C)).astype(np.float32),
        "agent_tokens": rng.standard_normal((B, 64, C)).astype(np.float32),
        "Wq": (rng.standard_normal((C, C)) * C ** -0.5).astype(np.float32),
        "Wk": (rng.standard_normal((C, C)) * C ** -0.5).astype(np.float32),
        "Wv": (rng.standard_normal((C, C)) * C ** -0.5).astype(np.float32),
        "Wp": (rng.standard_normal((C, C)) * C ** -0.5).astype(np.float32),
        "bp": (rng.standard_normal((C,)) * 0.01).astype(np.float32),
    }
    out = kernel(**ins)
    print("out", out.shape, out.dtype)


# revision 12
# speedup vs baseline: 1.0147x; 1.0130x over previous
"""AgentAttention Trainium2 kernel (8 NeuronCores).

Sharding: core c -> (batch b = c//2, head-group g = c%2). Each core computes
its batch's QKV projections for its 8 heads (512 channels), both attention
stages, and a partial output projection over its 512 channels for all 4096
tokens. The host sums the two per-batch partials and adds the bias.

Layout strategy: activations flow channel-major ("transposed", [C, T]) so
every matmul contraction sits on the partition axis with no on-chip
transposes. Softmaxes skip max-subtraction (logits are O(6) for this
problem's distribution); row sums come from ones-columns/ones-matmuls and
the per-(head, token) stage-2 normalizer is broadcast across partitions
with a selection matmul. All matmul operands are float32r (~1e-4 relative
error, 4x the float32 PE rate at moving-dim >= 256).

Heads are processed in pairs: the two heads' [64, d] agent blocks sit in a
block-diagonal [128, 128] operand so every PE op runs with full 128-wide
contraction/output. Stage-1's attn @ v runs per pair against a [t, 258]
slab holding two pairs' v channels plus a ones column (the l1 row-sum
rides along as output column 256).

Emission is breadth-first and pass 2 is software-pipelined
(stage2(b) -> q-proj(b+1) -> y-proj(b)) so the in-order PE stream always
has dense matmul work while ACT/DVE chase the exp/copy chain.
"""

import os
import sys

import numpy as np
import ml_dtypes

for _p in ("/opt/trn_rl_repo", "/root/.axon_site/_ro/trn_rl_repo"):
    if os.path.isdir(_p) and _p not in sys.path:
        sys.path.append(_p)

import concourse.tile as tile
from concourse import bacc, mybir
from concourse.bass_utils import run_bass_kernel_spmd

f32 = mybir.dt.float32
f32r = mybir.dt.float32r
bf16 = mybir.dt.bfloat16
Exp = mybir.ActivationFunctionType.Exp

B, N, C = 4, 4096, 1024
H, D = 16, 64
G = 2                 # head-groups (2-way tensor parallel x 4-way batch parallel)
GC = C // G           # 512 channels per group
NPAIR = GC // 128     # 4 head-pairs per group
TB = 512              # token block
NB = N // TB
KO = C // 128         # k-tiles over the 1024 input channels
AVW = 258             # stage-1 A_v slab: 2 pairs x 128 channels + ones + pad
SCALE = D ** -0.5

_cache = {}


def _build():
    if "nc" in _cache:
        return _cache["nc"]
    nc = bacc.Bacc("TRN2", target_bir_lowering=False, debug=False)

    xT_d = nc.dram_tensor("xT", [C, N], f32, kind="ExternalInput")
    wq_d = nc.dram_tensor("wqT", [C, GC], f32, kind="ExternalInput")
    wk_d = nc.dram_tensor("wkT", [C, GC], f32, kind="ExternalInput")
    wv_d = nc.dram_tensor("wvT", [C, GC], f32, kind="ExternalInput")
    wp_d = nc.dram_tensor("wpT", [GC, C], f32, kind="ExternalInput")
    ab_d = nc.dram_tensor("ablk", [NPAIR, 128, 128], f32, kind="ExternalInput")
    ones2_d = nc.dram_tensor("ones2", [128, 2], f32, kind="ExternalInput")
    sel2_d = nc.dram_tensor("sel2", [2, 128], f32, kind="ExternalInput")
    zeros_d = nc.dram_tensor("zeros", [128, NPAIR * 128], f32, kind="ExternalInput")
    vones_d = nc.dram_tensor("vones", [128, 4, 2, 2], f32, kind="ExternalInput")
    ident_d = nc.dram_tensor("ident", [128, 128], f32, kind="ExternalInput")
    y_d = nc.dram_tensor("y", [N, C], f32, kind="ExternalOutput")

    xT_r = xT_d.ap().bitcast(f32r).rearrange("(ko p) t -> p ko t", p=128)
    wq_r = wq_d.ap().bitcast(f32r).rearrange("(ko p) m -> p ko m", p=128)
    wk_r = wk_d.ap().bitcast(f32r).rearrange("(ko p) m -> p ko m", p=128)
    wv_r = wv_d.ap().bitcast(f32r).rearrange("(ko p) m -> p ko m", p=128)
    wp_r = wp_d.ap().bitcast(f32r).rearrange("(ko p) j -> p ko j", p=128)
    ab_r = ab_d.ap().bitcast(f32r).rearrange("h p a -> p h a")
    zeros_r = zeros_d.ap().bitcast(f32r).rearrange("p (h a) -> p h a", h=NPAIR)

    with tile.TileContext(nc) as tc:
        with (
            tc.tile_pool(name="wpool", bufs=1) as wpool,
            tc.tile_pool(name="cpool", bufs=1) as cpool,
            tc.tile_pool(name="accp", bufs=1) as accp,
        ):
            # weights + constants ride the ACT HWDGE ring so the SP ring can
            # start streaming x immediately
            wk_t = wpool.tile([128, KO, GC], f32r)
            wv_t = wpool.tile([128, KO, GC], f32r)
            wq_t = wpool.tile([128, KO, GC], f32r)
            wp_t = wpool.tile([128, NPAIR, C], f32r)
            for ko in range(KO):
                nc.scalar.dma_start(wk_t[:, ko, :], wk_r[:, ko, :])
            nc.scalar.dma_start(wv_t[:], wv_r)
            ab_t = cpool.tile([128, NPAIR, 128], f32r)
            nc.scalar.dma_start(ab_t[:], ab_r)
            ones2_t = cpool.tile([128, 2], f32r)
            nc.scalar.dma_start(ones2_t[:], ones2_d.ap().bitcast(f32r))
            sel2_t = cpool.tile([2, 128], f32r)
            nc.scalar.dma_start(sel2_t[:], sel2_d.ap().bitcast(f32r))
            ident_t = cpool.tile([128, 128], f32r)
            nc.scalar.dma_start(ident_t[:], ident_d.ap().bitcast(f32r))
            avacc = accp.tile([128, NPAIR, AVW], f32)
            avm = accp.tile([128, NPAIR, 128], f32r)
            nc.scalar.dma_start(avm[:], zeros_r)

            # ---------------- pass 1: k/v projections + stage-1 attention ----
            with (
                tc.tile_pool(name="xp1", bufs=2) as xp1,
                tc.tile_pool(name="kvp", bufs=2) as kvp,
                tc.tile_pool(name="pp1", bufs=8) as pp1,
                tc.tile_pool(name="psA", bufs=2, space="PSUM") as psA,
                tc.tile_pool(name="psB", bufs=4, space="PSUM") as psB,
                tc.tile_pool(name="psC", bufs=2, space="PSUM") as psC,
            ):
                for tb in range(NB):
                    xblk = xp1.tile([128, KO, TB], f32r, tag="xblk")
                    xsrc = xT_r[:, :, tb * TB:(tb + 1) * TB]
                    if tb == 0:
                        for ko in range(KO):
                            nc.sync.dma_start(xblk[:, ko, :], xsrc[:, ko, :])
                    else:
                        nc.sync.dma_start(xblk[:], xsrc)

                    # kT: [pair-channel, pair, tokens]
                    kT = kvp.tile([128, NPAIR, TB], f32r, tag="kT")
                    for m in range(NPAIR):
                        psk = psA.tile([128, TB], f32, tag="proj")
                        for ko in range(KO):
                            nc.tensor.matmul(
                                psk[:],
                                lhsT=wk_t[:, ko, m * 128:(m + 1) * 128],
                                rhs=xblk[:, ko, :],
                                start=(ko == 0), stop=(ko == KO - 1),
                            )
                        nc.scalar.copy(kT[:, m, :], psk[:])

                    # v slabs: [token-in-chunk, chunk, pair-group, 256 ch|1|0]
                    vt = kvp.tile([128, 4, 2, AVW], f32r, tag="vt")
                    nc.sync.dma_start(vt[:, :, :, 256:258], vones_d.ap().bitcast(f32r))
                    for mt in range(4):
                        psv = psA.tile([128, GC], f32, tag="proj")
                        for ko in range(KO):
                            nc.tensor.matmul(
                                psv[:],
                                lhsT=xblk[:, ko, mt * 128:(mt + 1) * 128],
                                rhs=wv_t[:, ko, :],
                                start=(ko == 0), stop=(ko == KO - 1),
                            )
                        nc.vector.tensor_copy(vt[:, mt, 0, 0:256], psv[:, 0:256])
                        nc.vector.tensor_copy(vt[:, mt, 1, 0:256], psv[:, 256:512])

                    # stage 1: agents attend over this block's keys.
                    # breadth-first: the 4 s1 matmuls run dense while ACT exps
                    # drain them, then the 4 av matmuls accumulate.
                    for hp in range(NPAIR):
                        pts = []
                        for ch in range(4):
                            pss = psB.tile([128, 128], f32, tag="s1")
                            nc.tensor.matmul(
                                pss[:],
                                lhsT=kT[:, hp, ch * 128:(ch + 1) * 128],
                                rhs=ab_t[:, hp, :],
                                start=True, stop=True,
                            )
                            pt = pp1.tile([128, 128], f32r, tag="p1")
                            nc.scalar.activation(pt[:], pss[:], Exp, scale=SCALE)
                            pts.append(pt)
                        psav = psC.tile([128, AVW], f32, tag="av")
                        for ch in range(4):
                            nc.tensor.matmul(
                                psav[:], lhsT=pts[ch][:],
                                rhs=vt[:, ch, hp // 2, :],
                                start=(ch == 0), stop=(ch == 3),
                            )
                        if tb == 0:
                            nc.vector.tensor_copy(avacc[:, hp, :], psav[:])
                        else:
                            nc.vector.tensor_add(avacc[:, hp, :], avacc[:, hp, :], psav[:])

                    if tb == 2:
                        # q/p weights arrive mid-pass-1: HBM is quiet here and
                        # they are not needed until pass 2
                        nc.scalar.dma_start(wq_t[:], wq_r)
                        nc.scalar.dma_start(wp_t[:], wp_r)

            # ---------------- pass 2: q projection + stage-2 + output proj ---
            with (
                tc.tile_pool(name="xp2", bufs=2) as xp2,
                tc.tile_pool(name="qp", bufs=2) as qp,
                tc.tile_pool(name="pp2", bufs=4) as pp2,
                tc.tile_pool(name="op", bufs=3) as op,
                tc.tile_pool(name="yp", bufs=4) as yp,
                tc.tile_pool(name="psD", bufs=2, space="PSUM") as psD,
                tc.tile_pool(name="psY", bufs=2, space="PSUM") as psY,
                tc.tile_pool(name="psSBO", bufs=3, space="PSUM") as psSBO,
                tc.tile_pool(name="psF", bufs=1, space="PSUM") as psF,
            ):
                def emit_qproj(tb):
                    xblk = xp2.tile([128, KO, TB], f32r, tag="xblk2")
                    nc.sync.dma_start(xblk[:], xT_r[:, :, tb * TB:(tb + 1) * TB])
                    qT = qp.tile([128, NPAIR, TB], f32r, tag="qT")
                    for m in range(NPAIR):
                        psq = psD.tile([128, TB], f32, tag="proj2")
                        for ko in range(KO):
                            nc.tensor.matmul(
                                psq[:],
                                lhsT=wq_t[:, ko, m * 128:(m + 1) * 128],
                                rhs=xblk[:, ko, :],
                                start=(ko == 0), stop=(ko == KO - 1),
                            )
                        nc.vector.tensor_copy(qT[:, m, :], psq[:])
                    return qT

                def emit_stage2(qT):
                    outT = op.tile([128, NPAIR, TB], f32r, tag="outT")
                    p2s, r2rs = [], []
                    for hp in range(NPAIR):
                        ps2 = psSBO.tile([128, TB], f32, tag="sbo")
                        nc.tensor.matmul(ps2[:], lhsT=ab_t[:, hp, :],
                                         rhs=qT[:, hp, :], start=True, stop=True)
                        p2 = pp2.tile([128, TB], f32r, tag="p2")
                        nc.scalar.activation(p2[:], ps2[:], Exp, scale=SCALE)
                        p2s.append(p2)
                    for hp in range(NPAIR):
                        psl2 = psF.tile([2, TB], f32, tag="l2")
                        nc.tensor.matmul(psl2[:], lhsT=ones2_t[:], rhs=p2s[hp][:],
                                         start=True, stop=True)
                        r2 = pp2.tile([2, TB], f32, tag="r2")
                        r2w = pp2.tile([2, TB], f32, tag="r2w")
                        nc.vector.reciprocal_approx_accurate(r2[:], psl2[:], r2w[:])
                        r2r = pp2.tile([2, TB], f32r, tag="r2r")
                        nc.vector.tensor_copy(r2r[:], r2[:])
                        r2rs.append(r2r)
                    for hp in range(NPAIR):
                        pso = psSBO.tile([128, TB], f32, tag="sbo")
                        nc.tensor.matmul(pso[:], lhsT=avm[:, hp, :], rhs=p2s[hp][:],
                                         start=True, stop=True)
                        psbc = psSBO.tile([128, TB], f32, tag="sbo")
                        nc.tensor.matmul(psbc[:], lhsT=sel2_t[:], rhs=r2rs[hp][:],
                                         start=True, stop=True)
                        bcs = pp2.tile([128, TB], f32, tag="bcs")
                        nc.scalar.copy(bcs[:], psbc[:])
                        with nc.allow_low_precision(reason="f32r rounding"):
                            nc.vector.tensor_mul(outT[:, hp, :], pso[:], bcs[:])
                    return outT

                def emit_yproj(tb, outT):
                    for tt in range(4):
                        for jn in range(2):
                            psy = psY.tile([128, 512], f32, tag="yproj")
                            for hp in range(NPAIR):
                                nc.tensor.matmul(
                                    psy[:],
                                    lhsT=outT[:, hp, tt * 128:(tt + 1) * 128],
                                    rhs=wp_t[:, hp, jn * 512:(jn + 1) * 512],
                                    start=(hp == 0), stop=(hp == NPAIR - 1),
                                )
                            ys = yp.tile([128, 512], f32, tag="ys")
                            if jn == 0:
                                nc.scalar.copy(ys[:], psy[:])
                            else:
                                nc.vector.tensor_copy(ys[:], psy[:])
                            row = tb * TB + tt * 128
                            nc.sync.dma_start(
                                y_d.ap()[row:row + 128, jn * 512:(jn + 1) * 512],
                                ys[:])

                qT = emit_qproj(0)

                # normalize agent_v, mask into block-diagonal f32r operand
                # (overlaps the q projection of block 0 on DVE)
                for hp in range(NPAIR):
                    hi = hp % 2
                    r1 = pp2.tile([128, 1], f32, tag="r1")
                    r1s = pp2.tile([128, 1], f32, tag="r1s")
                    nc.vector.reciprocal_approx_accurate(
                        r1[:], avacc[:, hp, 256:257], r1s[:])
                    with nc.allow_low_precision(reason="f32r rounding"):
                        nc.vector.tensor_scalar_mul(
                            avm[0:64, hp, 0:64],
                            avacc[0:64, hp, hi * 128:hi * 128 + 64],
                            r1[0:64, :])
                        nc.vector.tensor_scalar_mul(
                            avm[64:128, hp, 64:128],
                            avacc[64:128, hp, hi * 128 + 64:hi * 128 + 128],
                            r1[64:128, :])

                prev = None
                for tb in range(NB):
                    outT = emit_stage2(qT)
                    if tb + 1 < NB:
                        qT = emit_qproj(tb + 1)
                    if prev is not None:
                        emit_yproj(tb - 1, prev)
                    prev = outT
                emit_yproj(NB - 1, prev)

    nc.compile()
    _cache["nc"] = nc
    return nc


def _consts():
    ones2 = np.zeros((128, 2), np.float32)
    ones2[0:64, 0] = 1.0
    ones2[64:128, 1] = 1.0
    sel2 = np.zeros((2, 128), np.float32)
    sel2[0, 0:64] = 1.0
    sel2[1, 64:128] = 1.0
    zeros = np.zeros((128, NPAIR * 128), np.float32)
    vones = np.zeros((128, 4, 2, 2), np.float32)
    vones[:, :, :, 0] = 1.0
    ident = np.eye(128, dtype=np.float32)
    return ones2, sel2, zeros, vones, ident


def _in_maps(x, agent_tokens, Wq, Wk, Wv, Wp):
    ones2, sel2, zeros, vones, ident = _consts()
    xT = [np.ascontiguousarray(x[b].T) for b in range(B)]
    wqT = [np.ascontiguousarray(Wq[g * GC:(g + 1) * GC, :].T) for g in range(G)]
    wkT = [np.ascontiguousarray(Wk[g * GC:(g + 1) * GC, :].T) for g in range(G)]
    wvT = [np.ascontiguousarray(Wv[g * GC:(g + 1) * GC, :].T) for g in range(G)]
    wpT = [np.ascontiguousarray(Wp[:, g * GC:(g + 1) * GC].T) for g in range(G)]
    in_maps = []
    for core in range(8):
        b, g = core // 2, core % 2
        aT = agent_tokens[b][:, g * GC:(g + 1) * GC].T   # [512, 64]
        ablk = np.zeros((NPAIR, 128, 128), np.float32)
        for hp in range(NPAIR):
            ablk[hp, 0:64, 0:64] = aT[hp * 128:hp * 128 + 64, :]
            ablk[hp, 64:128, 64:128] = aT[hp * 128 + 64:hp * 128 + 128, :]
        in_maps.append({
            "xT": xT[b], "wqT": wqT[g], "wkT": wkT[g], "wvT": wvT[g],
            "wpT": wpT[g], "ablk": ablk, "ones2": ones2, "sel2": sel2,
            "zeros": zeros, "vones": vones, "ident": ident,
        })
    return in_maps


def kernel(x, agent_tokens, Wq, Wk, Wv, Wp, bp):
    x = np.asarray(x, dtype=np.float32)
    agent_tokens = np.asarray(agent_tokens, dtype=np.float32)
    Wq = np.asarray(Wq, dtype=np.float32)
    Wk = np.asarray(Wk, dtype=np.float32)
    Wv = np.asarray(Wv, dtype=np.float32)
    Wp = np.asarray(Wp, dtype=np.float32)
    bp = np.asarray(bp, dtype=np.float32)

    nc = _build()
    in_maps = _in_maps(x, agent_tokens, Wq, Wk, Wv, Wp)
    res = run_bass_kernel_spmd(nc, in_maps, list(range(8)))

    y = np.empty((B, N, C), np.float32)
    for b in range(B):
        y[b] = res.results[2 * b]["y"] + res.results[2 * b + 1]["y"] + bp[None, :]
    return y


if __name__ == "__main__":
    rng = np.random.default_rng(0)
    ins = {
        "x": rng.standard_normal((B, N, C)).astype(np.float32),
        "agent_tokens": rng.standard_normal((B, 64, C)).astype(np.float32),
        "Wq": (rng.standard_normal((C, C)) * C ** -0.5).astype(np.float32),
        "Wk": (rng.standard_normal((C, C)) * C ** -0.5).astype(np.float32),
        "Wv": (rng.standard_normal((C, C)) * C ** -0.5).astype(np.float32),
        "Wp": (rng.standard_normal((C, C)) * C ** -0.5).astype(np.float32),
        "bp": (rng.standard_normal((C,)) * 0.01).astype(np.float32),
    }
    out = kernel(**ins)
    print("out", out.shape, out.dtype)


# revision 13
# speedup vs baseline: 1.0332x; 1.0182x over previous
"""AgentAttention Trainium2 kernel (8 NeuronCores).

Sharding: core c -> (batch b = c//2, head-group g = c%2). Each core computes
its batch's QKV projections for its 8 heads (512 channels), both attention
stages, and a partial output projection over its 512 channels for all 4096
tokens. The host sums the two per-batch partials and adds the bias.

Layout strategy: activations flow channel-major ("transposed", [C, T]) so
every matmul contraction sits on the partition axis with no on-chip
transposes. Softmaxes skip max-subtraction (logits are O(6) for this
problem's distribution); row sums come from ones-columns/ones-matmuls and
the per-(head, token) stage-2 normalizer is broadcast across partitions
with a selection matmul. All matmul operands are float32r (~1e-4 relative
error, 4x the float32 PE rate at moving-dim >= 256).

Heads are processed in pairs: the two heads' [64, d] agent blocks sit in a
block-diagonal [128, 128] operand so every PE op runs with full 128-wide
contraction/output. Stage-1's attn @ v runs per pair against a [t, 258]
slab holding two pairs' v channels plus a ones column (the l1 row-sum
rides along as output column 256).

Emission is breadth-first and pass 2 is software-pipelined
(stage2(b) -> q-proj(b+1) -> y-proj(b)) so the in-order PE stream always
has dense matmul work while ACT/DVE chase the exp/copy chain.
"""

import os
import sys

import numpy as np
import ml_dtypes

for _p in ("/opt/trn_rl_repo", "/root/.axon_site/_ro/trn_rl_repo"):
    if os.path.isdir(_p) and _p not in sys.path:
        sys.path.append(_p)

import concourse.tile as tile
from concourse import bacc, mybir
from concourse.bass_utils import run_bass_kernel_spmd

f32 = mybir.dt.float32
f32r = mybir.dt.float32r
bf16 = mybir.dt.bfloat16
Exp = mybir.ActivationFunctionType.Exp

B, N, C = 4, 4096, 1024
H, D = 16, 64
G = 2                 # head-groups (2-way tensor parallel x 4-way batch parallel)
GC = C // G           # 512 channels per group
NPAIR = GC // 128     # 4 head-pairs per group
TB = 512              # token block
NB = N // TB
KO = C // 128         # k-tiles over the 1024 input channels
AVW = 258             # stage-1 A_v slab: 2 pairs x 128 channels + ones + pad
SCALE = D ** -0.5

_cache = {}


def _build():
    if "nc" in _cache:
        return _cache["nc"]
    nc = bacc.Bacc("TRN2", target_bir_lowering=False, debug=False)

    xT_d = nc.dram_tensor("xT", [C, N], f32, kind="ExternalInput")
    wq_d = nc.dram_tensor("wqT", [C, GC], f32, kind="ExternalInput")
    wk_d = nc.dram_tensor("wkT", [C, GC], f32, kind="ExternalInput")
    wv_d = nc.dram_tensor("wvT", [C, GC], f32, kind="ExternalInput")
    wp_d = nc.dram_tensor("wpT", [GC, C], f32, kind="ExternalInput")
    ab_d = nc.dram_tensor("ablk", [NPAIR, 128, 128], f32, kind="ExternalInput")
    ones2_d = nc.dram_tensor("ones2", [128, 2], f32, kind="ExternalInput")
    sel2_d = nc.dram_tensor("sel2", [2, 128], f32, kind="ExternalInput")
    zeros_d = nc.dram_tensor("zeros", [128, NPAIR * 128], f32, kind="ExternalInput")
    vones_d = nc.dram_tensor("vones", [128, 4, 2, 2], f32, kind="ExternalInput")
    ident_d = nc.dram_tensor("ident", [128, 128], f32, kind="ExternalInput")
    y_d = nc.dram_tensor("y", [N, C], f32, kind="ExternalOutput")

    xT_r = xT_d.ap().bitcast(f32r).rearrange("(ko p) t -> p ko t", p=128)
    wq_r = wq_d.ap().bitcast(f32r).rearrange("(ko p) m -> p ko m", p=128)
    wk_r = wk_d.ap().bitcast(f32r).rearrange("(ko p) m -> p ko m", p=128)
    wv_r = wv_d.ap().bitcast(f32r).rearrange("(ko p) m -> p ko m", p=128)
    wp_r = wp_d.ap().bitcast(f32r).rearrange("(ko p) j -> p ko j", p=128)
    ab_r = ab_d.ap().bitcast(f32r).rearrange("h p a -> p h a")
    zeros_r = zeros_d.ap().bitcast(f32r).rearrange("p (h a) -> p h a", h=NPAIR)

    with tile.TileContext(nc) as tc:
        with (
            tc.tile_pool(name="wpool", bufs=1) as wpool,
            tc.tile_pool(name="cpool", bufs=1) as cpool,
            tc.tile_pool(name="accp", bufs=1) as accp,
        ):
            # weights + constants ride the ACT HWDGE ring so the SP ring can
            # start streaming x immediately
            wk_t = wpool.tile([128, KO, GC], f32r)
            wv_t = wpool.tile([128, KO, GC], f32r)
            wq_t = wpool.tile([128, KO, GC], f32r)
            wp_t = wpool.tile([128, NPAIR, C], f32r)
            ident_t0 = cpool.tile([128, 128], f32r)
            nc.scalar.dma_start(ident_t0[:], ident_d.ap().bitcast(f32r))
            for ko in range(KO):
                nc.scalar.dma_start(wk_t[:, ko, :], wk_r[:, ko, :])
            nc.scalar.dma_start(wv_t[:], wv_r)
            ab_t = cpool.tile([128, NPAIR, 128], f32r)
            nc.scalar.dma_start(ab_t[:], ab_r)
            ones2_t = cpool.tile([128, 2], f32r)
            nc.scalar.dma_start(ones2_t[:], ones2_d.ap().bitcast(f32r))
            sel2_t = cpool.tile([2, 128], f32r)
            nc.scalar.dma_start(sel2_t[:], sel2_d.ap().bitcast(f32r))
            avacc = accp.tile([128, NPAIR, AVW], f32)
            avm = accp.tile([128, NPAIR, 128], f32r)
            nc.scalar.dma_start(avm[:], zeros_r)

            # ---------------- pass 1: k/v projections + stage-1 attention ----
            with (
                tc.tile_pool(name="xp1", bufs=2) as xp1,
                tc.tile_pool(name="kvp", bufs=2) as kvp,
                tc.tile_pool(name="pp1", bufs=8) as pp1,
                tc.tile_pool(name="psA", bufs=2, space="PSUM") as psA,
                tc.tile_pool(name="psB", bufs=4, space="PSUM") as psB,
                tc.tile_pool(name="psC", bufs=2, space="PSUM") as psC,
            ):
                def warm_pe(n):
                    for _ in range(n):
                        psw = psB.tile([128, 128], f32, tag="s1")
                        nc.tensor.matmul(psw[:], lhsT=ident_t0[:],
                                         rhs=ident_t0[:], start=True, stop=True)

                warm_pe(10)
                for tb in range(NB):
                    xblk = xp1.tile([128, KO, TB], f32r, tag="xblk")
                    xsrc = xT_r[:, :, tb * TB:(tb + 1) * TB]
                    if tb == 0:
                        for ko in range(KO):
                            nc.sync.dma_start(xblk[:, ko, :], xsrc[:, ko, :])
                    else:
                        nc.sync.dma_start(xblk[:], xsrc)
                    if tb == 1:
                        warm_pe(10)

                    # kT: [pair-channel, pair, tokens]
                    kT = kvp.tile([128, NPAIR, TB], f32r, tag="kT")
                    for m in range(NPAIR):
                        psk = psA.tile([128, TB], f32, tag="proj")
                        for ko in range(KO):
                            nc.tensor.matmul(
                                psk[:],
                                lhsT=wk_t[:, ko, m * 128:(m + 1) * 128],
                                rhs=xblk[:, ko, :],
                                start=(ko == 0), stop=(ko == KO - 1),
                            )
                        nc.scalar.copy(kT[:, m, :], psk[:])

                    # v slabs: [token-in-chunk, chunk, pair-group, 256 ch|1|0]
                    vt = kvp.tile([128, 4, 2, AVW], f32r, tag="vt")
                    nc.sync.dma_start(vt[:, :, :, 256:258], vones_d.ap().bitcast(f32r))
                    for mt in range(4):
                        psv = psA.tile([128, GC], f32, tag="proj")
                        for ko in range(KO):
                            nc.tensor.matmul(
                                psv[:],
                                lhsT=xblk[:, ko, mt * 128:(mt + 1) * 128],
                                rhs=wv_t[:, ko, :],
                                start=(ko == 0), stop=(ko == KO - 1),
                            )
                        nc.vector.tensor_copy(vt[:, mt, 0, 0:256], psv[:, 0:256])
                        nc.vector.tensor_copy(vt[:, mt, 1, 0:256], psv[:, 256:512])

                    # stage 1: agents attend over this block's keys.
                    # breadth-first: the 4 s1 matmuls run dense while ACT exps
                    # drain them, then the 4 av matmuls accumulate.
                    for hp in range(NPAIR):
                        pts = []
                        for ch in range(4):
                            pss = psB.tile([128, 128], f32, tag="s1")
                            nc.tensor.matmul(
                                pss[:],
                                lhsT=kT[:, hp, ch * 128:(ch + 1) * 128],
                                rhs=ab_t[:, hp, :],
                                start=True, stop=True,
                            )
                            pt = pp1.tile([128, 128], f32r, tag="p1")
                            nc.scalar.activation(pt[:], pss[:], Exp, scale=SCALE)
                            pts.append(pt)
                        psav = psC.tile([128, AVW], f32, tag="av")
                        for ch in range(4):
                            nc.tensor.matmul(
                                psav[:], lhsT=pts[ch][:],
                                rhs=vt[:, ch, hp // 2, :],
                                start=(ch == 0), stop=(ch == 3),
                            )
                        if tb == 0:
                            nc.vector.tensor_copy(avacc[:, hp, :], psav[:])
                        else:
                            nc.vector.tensor_add(avacc[:, hp, :], avacc[:, hp, :], psav[:])

                    if tb == 2:
                        # q/p weights arrive mid-pass-1: HBM is quiet here and
                        # they are not needed until pass 2
                        nc.scalar.dma_start(wq_t[:], wq_r)
                        nc.scalar.dma_start(wp_t[:], wp_r)

            # ---------------- pass 2: q projection + stage-2 + output proj ---
            with (
                tc.tile_pool(name="xp2", bufs=2) as xp2,
                tc.tile_pool(name="qp", bufs=2) as qp,
                tc.tile_pool(name="pp2", bufs=4) as pp2,
                tc.tile_pool(name="op", bufs=3) as op,
                tc.tile_pool(name="yp", bufs=4) as yp,
                tc.tile_pool(name="psD", bufs=2, space="PSUM") as psD,
                tc.tile_pool(name="psY", bufs=2, space="PSUM") as psY,
                tc.tile_pool(name="psSBO", bufs=3, space="PSUM") as psSBO,
                tc.tile_pool(name="psF", bufs=1, space="PSUM") as psF,
            ):
                def emit_qproj(tb):
                    xblk = xp2.tile([128, KO, TB], f32r, tag="xblk2")
                    nc.sync.dma_start(xblk[:], xT_r[:, :, tb * TB:(tb + 1) * TB])
                    qT = qp.tile([128, NPAIR, TB], f32r, tag="qT")
                    for m in range(NPAIR):
                        psq = psD.tile([128, TB], f32, tag="proj2")
                        for ko in range(KO):
                            nc.tensor.matmul(
                                psq[:],
                                lhsT=wq_t[:, ko, m * 128:(m + 1) * 128],
                                rhs=xblk[:, ko, :],
                                start=(ko == 0), stop=(ko == KO - 1),
                            )
                        nc.vector.tensor_copy(qT[:, m, :], psq[:])
                    return qT

                def emit_stage2(qT):
                    outT = op.tile([128, NPAIR, TB], f32r, tag="outT")
                    p2s, r2rs = [], []
                    for hp in range(NPAIR):
                        ps2 = psSBO.tile([128, TB], f32, tag="sbo")
                        nc.tensor.matmul(ps2[:], lhsT=ab_t[:, hp, :],
                                         rhs=qT[:, hp, :], start=True, stop=True)
                        p2 = pp2.tile([128, TB], f32r, tag="p2")
                        nc.scalar.activation(p2[:], ps2[:], Exp, scale=SCALE)
                        p2s.append(p2)
                    for hp in range(NPAIR):
                        psl2 = psF.tile([2, TB], f32, tag="l2")
                        nc.tensor.matmul(psl2[:], lhsT=ones2_t[:], rhs=p2s[hp][:],
                                         start=True, stop=True)
                        r2 = pp2.tile([2, TB], f32, tag="r2")
                        r2w = pp2.tile([2, TB], f32, tag="r2w")
                        nc.vector.reciprocal_approx_accurate(r2[:], psl2[:], r2w[:])
                        r2r = pp2.tile([2, TB], f32r, tag="r2r")
                        nc.vector.tensor_copy(r2r[:], r2[:])
                        r2rs.append(r2r)
                    for hp in range(NPAIR):
                        pso = psSBO.tile([128, TB], f32, tag="sbo")
                        nc.tensor.matmul(pso[:], lhsT=avm[:, hp, :], rhs=p2s[hp][:],
                                         start=True, stop=True)
                        psbc = psSBO.tile([128, TB], f32, tag="sbo")
                        nc.tensor.matmul(psbc[:], lhsT=sel2_t[:], rhs=r2rs[hp][:],
                                         start=True, stop=True)
                        bcs = pp2.tile([128, TB], f32, tag="bcs")
                        nc.scalar.copy(bcs[:], psbc[:])
                        with nc.allow_low_precision(reason="f32r rounding"):
                            nc.vector.tensor_mul(outT[:, hp, :], pso[:], bcs[:])
                    return outT

                def emit_yproj(tb, outT):
                    for tt in range(4):
                        for jn in range(2):
                            psy = psY.tile([128, 512], f32, tag="yproj")
                            for hp in range(NPAIR):
                                nc.tensor.matmul(
                                    psy[:],
                                    lhsT=outT[:, hp, tt * 128:(tt + 1) * 128],
                                    rhs=wp_t[:, hp, jn * 512:(jn + 1) * 512],
                                    start=(hp == 0), stop=(hp == NPAIR - 1),
                                )
                            ys = yp.tile([128, 512], f32, tag="ys")
                            if jn == 0:
                                nc.scalar.copy(ys[:], psy[:])
                            else:
                                nc.vector.tensor_copy(ys[:], psy[:])
                            row = tb * TB + tt * 128
                            nc.sync.dma_start(
                                y_d.ap()[row:row + 128, jn * 512:(jn + 1) * 512],
                                ys[:])

                qT = emit_qproj(0)

                # normalize agent_v, mask into block-diagonal f32r operand
                # (overlaps the q projection of block 0 on DVE)
                for hp in range(NPAIR):
                    hi = hp % 2
                    r1 = pp2.tile([128, 1], f32, tag="r1")
                    r1s = pp2.tile([128, 1], f32, tag="r1s")
                    nc.vector.reciprocal_approx_accurate(
                        r1[:], avacc[:, hp, 256:257], r1s[:])
                    with nc.allow_low_precision(reason="f32r rounding"):
                        nc.vector.tensor_scalar_mul(
                            avm[0:64, hp, 0:64],
                            avacc[0:64, hp, hi * 128:hi * 128 + 64],
                            r1[0:64, :])
                        nc.vector.tensor_scalar_mul(
                            avm[64:128, hp, 64:128],
                            avacc[64:128, hp, hi * 128 + 64:hi * 128 + 128],
                            r1[64:128, :])

                prev = None
                for tb in range(NB):
                    outT = emit_stage2(qT)
                    if tb + 1 < NB:
                        qT = emit_qproj(tb + 1)
                    if prev is not None:
                        emit_yproj(tb - 1, prev)
                    prev = outT
                emit_yproj(NB - 1, prev)

    nc.compile()
    _cache["nc"] = nc
    return nc


def _consts():
    ones2 = np.zeros((128, 2), np.float32)
    ones2[0:64, 0] = 1.0
    ones2[64:128, 1] = 1.0
    sel2 = np.zeros((2, 128), np.float32)
    sel2[0, 0:64] = 1.0
    sel2[1, 64:128] = 1.0
    zeros = np.zeros((128, NPAIR * 128), np.float32)
    vones = np.zeros((128, 4, 2, 2), np.float32)
    vones[:, :, :, 0] = 1.0
    ident = np.eye(128, dtype=np.float32)
    return ones2, sel2, zeros, vones, ident


def _in_maps(x, agent_tokens, Wq, Wk, Wv, Wp):
    ones2, sel2, zeros, vones, ident = _consts()
    xT = [np.ascontiguousarray(x[b].T) for b in range(B)]
    wqT = [np.ascontiguousarray(Wq[g * GC:(g + 1) * GC, :].T) for g in range(G)]
    wkT = [np.ascontiguousarray(Wk[g * GC:(g + 1) * GC, :].T) for g in range(G)]
    wvT = [np.ascontiguousarray(Wv[g * GC:(g + 1) * GC, :].T) for g in range(G)]
    wpT = [np.ascontiguousarray(Wp[:, g * GC:(g + 1) * GC].T) for g in range(G)]
    in_maps = []
    for core in range(8):
        b, g = core // 2, core % 2
        aT = agent_tokens[b][:, g * GC:(g + 1) * GC].T   # [512, 64]
        ablk = np.zeros((NPAIR, 128, 128), np.float32)
        for hp in range(NPAIR):
            ablk[hp, 0:64, 0:64] = aT[hp * 128:hp * 128 + 64, :]
            ablk[hp, 64:128, 64:128] = aT[hp * 128 + 64:hp * 128 + 128, :]
        in_maps.append({
            "xT": xT[b], "wqT": wqT[g], "wkT": wkT[g], "wvT": wvT[g],
            "wpT": wpT[g], "ablk": ablk, "ones2": ones2, "sel2": sel2,
            "zeros": zeros, "vones": vones, "ident": ident,
        })
    return in_maps


def kernel(x, agent_tokens, Wq, Wk, Wv, Wp, bp):
    x = np.asarray(x, dtype=np.float32)
    agent_tokens = np.asarray(agent_tokens, dtype=np.float32)
    Wq = np.asarray(Wq, dtype=np.float32)
    Wk = np.asarray(Wk, dtype=np.float32)
    Wv = np.asarray(Wv, dtype=np.float32)
    Wp = np.asarray(Wp, dtype=np.float32)
    bp = np.asarray(bp, dtype=np.float32)

    nc = _build()
    in_maps = _in_maps(x, agent_tokens, Wq, Wk, Wv, Wp)
    res = run_bass_kernel_spmd(nc, in_maps, list(range(8)))

    y = np.empty((B, N, C), np.float32)
    for b in range(B):
        y[b] = res.results[2 * b]["y"] + res.results[2 * b + 1]["y"] + bp[None, :]
    return y


if __name__ == "__main__":
    rng = np.random.default_rng(0)
    ins = {
        "x": rng.standard_normal((B, N, C)).astype(np.float32),
        "agent_tokens": rng.standard_normal((B, 64, C)).astype(np.float32),
        "Wq": (rng.standard_normal((C, C)) * C ** -0.5).astype(np.float32),
        "Wk": (rng.standard_normal((C, C)) * C ** -0.5).astype(np.float32),
        "Wv": (rng.standard_normal((C, C)) * C ** -0.5).astype(np.float32),
        "Wp": (rng.standard_normal((C, C)) * C ** -0.5).astype(np.float32),
        "bp": (rng.standard_normal((C,)) * 0.01).astype(np.float32),
    }
    out = kernel(**ins)
    print("out", out.shape, out.dtype)
